# revision 40
# baseline (speedup 1.0000x reference)
"""Trainium2 Bass kernel for nn_ComputeEdgeLoss.

Computes, for each batch b and lower-triangular pair (i, j) of the 64
recon keypoints, the mean over 5 interpolated segment points of the min
squared distance to the 2048 gt points of that batch.

Strategy (v2.1)
---------------
Sharding: 8 cores = 4 batches x 2 pair-halves (1008 pairs each); the 64
endpoint queries of a batch are split 32/32 between its two cores, so
each core owns exactly 3056 query rows -> 24 row-tiles of 128 (the
baseline's 25).  gt replicated per batch (sharding_hint).

Math: for a query point k and gt point g,
    ||k - g||^2 = a . b,  a = [kx, ky, kz, ||k||^2, 1],
                          b = [-2gx, -2gy, -2gz, 1, ||g||^2]
so one PE matmul produces a [128 x 512] block of squared distances in
PSUM.  Precision: fp32 inputs are split into three bf16 terms
x ~= h + l + r and the six product groups >= 2^-24 (hh, hl, lh, hr, rh,
ll) become K=30 bf16 contraction rows (padded to 32).  Measured on the
real data this lands at ~3e-4 max rel err vs the 2e-2 gate.

PE array packing: K=32 <= 32 enables 4x row tiling -- the 128x128 PE
splits into four 32x128 bands (tile_position=(32q, 0)), each holding a
different stationary pair-block and streaming its own gt chunk, so four
matmuls run concurrently (~3x measured PE throughput).  Operands for
band q live on SBUF partitions [32q, 32q+32): gt is replicated at all
four offsets, pair tiles at the two offsets their halves use.

Reduction (the bottleneck): this walrus rejects every raw-ISA DVE
instruction ("ISA wrong length" in visitInstISA) -- no custom DVE ops,
no InstTensorTensorReduce -- so the drain uses only classic BIR ops:
 - A-tiles: DVE fp32 tensor_reduce(min) straight from PSUM (1 el/cyc);
 - B-tiles: ScalarE casts the [128, 2048] tile to fp16 in SBUF
   (1 el/cyc), DVE folds KB=3 staged tiles with 2x-rate fp16
   tensor_tensor(min) chains + one batched tensor_reduce.
6 A + 18 B balances DVE (~34us) against ScalarE (~33us); PE ~7-14us
is fully hidden behind the drain.

v2.1 deltas (structure-preserving): the A-tile of each group of 4 comes
FIRST (A_SET at 0,4,...) so the DVE has PSUM work as soon as tile 0
lands instead of idling ~16us behind three ScalarE copies; and a 1-col
warmup copy pulls the one-time ~1.3us ACT table load off the ScalarE
critical path.
"""

import numpy as np

import concourse.bass as bass
import concourse.mybir as mybir
import concourse.tile as tile
from concourse.bass_utils import run_bass_kernel_spmd

# Problem shape (hardcoded per contest rules).
B = 4          # batches
NPTS = 64      # recon points per batch
M = 2048       # gt points per batch
P = NPTS * (NPTS - 1) // 2   # 2016 pairs
HALF = P // 2                # 1008 pairs per core
N_CORES = 8
FRACS = (0.25, 0.5, 0.75)    # interior interpolation fractions
NF = len(FRACS)
NINT = NF * HALF             # 3024 interior rows per core
NEND = 32                    # endpoint rows per core (64 split 32/32)
NTILES = 24                  # row-tiles of 128 (3072 rows, 3056 used)
ROWS = NTILES * 128
KEXT = 32                    # padded contraction depth (30 real rows)
PFCOLS = (NTILES // 2) * 128  # 1536: even tiles on partitions 0-63, odd on 64-127
ABCOLS = M + PFCOLS

_II, _JJ = np.tril_indices(NPTS, -1)   # pair order matches reference

# Tiles drained by DVE directly (A); the rest are ScalarE-staged (B).
# Pattern A,B,B,B per group of 4: the A-tile gives the DVE PSUM work the
# moment the first tile lands; the three staged copies then chain
# back-to-back on ScalarE (each buffer refills during the next copy) and
# the DVE folds run one group behind the copies.
A_SET = (0, 4, 8, 12, 16, 20)
KB = 3                       # B-tiles per batched DVE fold chain


def _split3_bf16(x: np.ndarray):
    """Split fp32 x into three bf16 terms with x ~= h + l + r (27-bit
    significand fidelity; differences are Sterbenz-exact in fp32)."""
    import ml_dtypes

    bf16 = ml_dtypes.bfloat16
    x = np.ascontiguousarray(x, dtype=np.float32)
    h = x.astype(bf16)
    l32 = (x - h.astype(np.float32)).astype(np.float32)
    l = l32.astype(bf16)
    r = (l32 - l.astype(np.float32)).astype(np.float32).astype(bf16)
    return h, l, r


_COMPUTE_ENGINES = {"PE", "DVE", "Activation", "Pool"}


def _prune_redundant_waits(bir: dict) -> dict:
    """Reduce every instruction to at most ONE sync-wait.

    This walrus build accepts only one sync-wait per instruction, but
    Tile's semaphore pass is not transitively minimal.  We reconstruct
    per-instruction guaranteed semaphore lower bounds (vector clocks
    over the scheduled program order) and delete implied waits; any
    residual multi-wait instruction is split into single-wait Drain
    carriers on the same engine.

    Soundness model: per-engine in-order dispatch; in-order completion
    for compute engines; per-semaphore in-order completion for DMA-queue
    sems (each DMAHW sem belongs to one queue).  Only monotone
    (inc-only) semaphores with sem-ge-imm waits participate.
    """
    fn = bir["functions"][0]

    # Semaphore properties across the whole program.
    contrib_engines: dict[int, set] = {}
    monotone: dict[int, bool] = {}
    for b in fn["blocks"]:
        for ins in b["instructions"]:
            sy = ins.get("sync_info") or {}
            for u in sy.get("on_update") or []:
                if u.get("sync_type") != "semaphore":
                    continue
                s = u["id"]
                contrib_engines.setdefault(s, set()).add(ins.get("engine"))
                ok = u.get("update_mode") == "sem-inc"
                monotone[s] = monotone.get(s, True) and ok

    def usable(s):
        return monotone.get(s, False)

    def mergemax(dst, src):
        for k, v in src.items():
            if dst.get(k, -1) < v:
                dst[k] = v

    prev_start_know: dict[str, dict] = {}
    cum: dict[int, int] = {}            # sem -> cumulative inc in walk order
    comp_know: list[dict] = []          # per walk index
    sem_reach: dict[int, list] = {}     # sem -> [(value_after, walk_idx)]
    dropped = 0
    walk_idx = 0

    for b in fn["blocks"]:
        new_insts = []
        for ins in b["instructions"]:
            eng = ins.get("engine")
            sy = ins.get("sync_info") or {}
            waits = list(sy.get("on_wait") or [])

            def know_from(wlist):
                know = dict(prev_start_know.get(eng, {}))
                for w in wlist:
                    if (w.get("sync_type") != "semaphore"
                            or w.get("wait_mode") != "sem-ge-imm"):
                        continue
                    s, v = w["id"], w["wait_value"]
                    if not usable(s):
                        continue
                    if know.get(s, -1) < v:
                        know[s] = v
                    if len(contrib_engines.get(s, ())) == 1:
                        for after, pidx in sem_reach.get(s, ()):
                            if after >= v:
                                mergemax(know, comp_know[pidx])
                                break
                return know

            if len(waits) > 1:
                kept = list(waits)
                changed = True
                while changed and len(kept) > 1:
                    changed = False
                    for w in list(kept):
                        others = [x for x in kept if x is not w]
                        if (w.get("sync_type") == "semaphore"
                                and w.get("wait_mode") == "sem-ge-imm"
                                and usable(w["id"])
                                and know_from(others).get(w["id"], -1)
                                >= w["wait_value"]):
                            kept.remove(w)
                            dropped += 1
                            changed = True
                            break
                if len(kept) > 1:
                    # Split: carrier Drains each take one wait.
                    for k, w in enumerate(kept[:-1]):
                        new_insts.append({
                            "name": f"{ins['name']}-w{k}",
                            "engine": eng, "ins": [], "outs": [],
                            "opcode": "Drain",
                            "sync_info": {"on_wait": [w], "on_update": []},
                        })
                        walk_idx += 1
                        comp_know.append(dict(prev_start_know.get(eng, {})))
                    kept = kept[-1:]
                if len(kept) != len(waits):
                    if not sy:
                        ins["sync_info"] = sy = {"on_update": []}
                    sy["on_wait"] = kept
                    waits = kept

            start_know = know_from(waits)
            prev_start_know[eng] = start_know

            own = set()
            for u in sy.get("on_update") or []:
                if (u.get("sync_type") == "semaphore"
                        and u.get("update_mode") == "sem-inc"):
                    s = u["id"]
                    cum[s] = cum.get(s, 0) + u.get("update_value", 1)
                    sem_reach.setdefault(s, []).append((cum[s], walk_idx))
                    own.add(s)
            ck = dict(start_know)
            for s in own:
                if usable(s) and len(contrib_engines.get(s, ())) == 1:
                    if ck.get(s, -1) < cum[s]:
                        ck[s] = cum[s]
            if eng in _COMPUTE_ENGINES:
                for s, c in cum.items():
                    if (usable(s) and contrib_engines.get(s) == {eng}
                            and ck.get(s, -1) < c):
                        ck[s] = c
            comp_know.append(ck)
            new_insts.append(ins)
            walk_idx += 1
        b["instructions"] = new_insts
    return bir


def _build_nc() -> bass.Bass:
    nc = bass.Bass()
    # Single fused input tensor: gt (replicated per 32-partition band)
    # in cols [0, M), pair tiles in cols [M, ABCOLS).
    ab = nc.declare_dram_parameter("ab", [128, ABCOLS], mybir.dt.bfloat16,
                                   isOutput=False)
    n_a = len(A_SET)
    n_b = NTILES - n_a
    assert n_b % KB == 0
    res = nc.declare_dram_parameter("res", [128, NTILES + len(A_SET)],
                                    mybir.dt.float32, isOutput=True)

    f32 = mybir.dt.float32
    bf16 = mybir.dt.bfloat16
    f16 = mybir.dt.float16

    with tile.TileContext(nc) as tc:
        with (
            tc.tile_pool(name="const", bufs=1) as const_pool,
            tc.tile_pool(name="psum", bufs=4, space="PSUM") as psum_pool,
            tc.tile_pool(name="cp", bufs=2) as cp_pool,
            tc.tile_pool(name="fold", bufs=2) as fold_pool,
        ):
            AB = const_pool.tile([128, ABCOLS], bf16, name="AB")
            AMINS = const_pool.tile([128, 2 * n_a], f32, name="AMINS")
            BMINS = const_pool.tile([128, n_b], f32, name="BMINS")
            import os as _os
            _ww = 1 + int(_os.environ.get("KERNEL_SALT", "0"))
            WARM = const_pool.tile([128, _ww], f32, name="WARM")

            # Pull the one-time ~1.3us ACT table load off the ScalarE
            # critical path: a 1-col warmup copy right after the engine
            # preamble, overlapping the input DMA.
            nc.gpsimd.memset(WARM[:, :], 0.0)
            nc.scalar.copy(WARM[:, :], WARM[:, :])

            # Full-partition-width loads (partition-sliced DMAs are slow),
            # ordered so tile 0's operands (gt lo-half + first pf block)
            # land first and its matmuls start as early as possible.
            PF0 = M + 4 * 128     # pf tiles 0..7 boundary
            nc.sync.dma_start(out=AB[:, 0:M // 2], in_=ab[:, 0:M // 2])
            nc.sync.dma_start(out=AB[:, M:PF0], in_=ab[:, M:PF0])
            nc.sync.dma_start(out=AB[:, M // 2:M], in_=ab[:, M // 2:M])
            nc.sync.dma_start(out=AB[:, PF0:ABCOLS], in_=ab[:, PF0:ABCOLS])

            a_idx = 0
            b_idx = 0
            cp_cur = None
            HM = M // 2
            for t in range(NTILES):
                col = M + (t // 2) * 128
                halves = []
                for hh in range(2):
                    q = (2 * t + hh) % 4
                    stat = AB[32 * q:32 * q + 32, col:col + 128]
                    ptile = psum_pool.tile([128, HM], f32, tag="ptile")
                    halves.append(ptile)
                    for c in range(2):
                        sl_g = slice(HM * hh + 512 * c, HM * hh + 512 * (c + 1))
                        nc.tensor.matmul(
                            out=ptile[:, 512 * c:512 * (c + 1)],
                            lhsT=stat, rhs=AB[32 * q:32 * q + 32, sl_g],
                            start=True, stop=True,
                            tile_position=(32 * q, 0),
                        )
                if t in A_SET:
                    for hh in range(2):
                        nc.vector.tensor_reduce(
                            out=AMINS[:, 2 * a_idx + hh:2 * a_idx + hh + 1],
                            in_=halves[hh][:, :],
                            axis=mybir.AxisListType.X, op=mybir.AluOpType.min,
                        )
                    a_idx += 1
                    continue

                j = b_idx % KB
                if j == 0:
                    cp_cur = cp_pool.tile([128, KB * M], f16, tag="cp")
                for hh in range(2):
                    nc.scalar.copy(
                        cp_cur[:, j * M + hh * HM:j * M + (hh + 1) * HM],
                        halves[hh][:, :])
                b_idx += 1
                if j == KB - 1:
                    b0 = b_idx - KB
                    c3 = cp_cur[:, :].rearrange("p (k n) -> p k n", n=M)
                    j1 = fold_pool.tile([128, KB * (M // 2)], f16, tag="j1")
                    v1 = j1[:, :].rearrange("p (k n) -> p k n", n=M // 2)
                    nc.vector.tensor_tensor(
                        out=v1, in0=c3[:, :, 0:M // 2], in1=c3[:, :, M // 2:M],
                        op=mybir.AluOpType.min)
                    j2 = fold_pool.tile([128, KB * (M // 4)], f16, tag="j2")
                    v2 = j2[:, :].rearrange("p (k n) -> p k n", n=M // 4)
                    nc.vector.tensor_tensor(
                        out=v2, in0=v1[:, :, 0:M // 4], in1=v1[:, :, M // 4:M // 2],
                        op=mybir.AluOpType.min)
                    j3 = fold_pool.tile([128, KB * (M // 8)], f16, tag="j3")
                    v3 = j3[:, :].rearrange("p (k n) -> p k n", n=M // 8)
                    nc.vector.tensor_tensor(
                        out=v3, in0=v2[:, :, 0:M // 8], in1=v2[:, :, M // 8:M // 4],
                        op=mybir.AluOpType.min)
                    j4 = fold_pool.tile([128, KB * (M // 16)], f16, tag="j4")
                    v4 = j4[:, :].rearrange("p (k n) -> p k n", n=M // 16)
                    nc.vector.tensor_tensor(
                        out=v4, in0=v3[:, :, 0:M // 16], in1=v3[:, :, M // 16:M // 8],
                        op=mybir.AluOpType.min)
                    nc.vector.tensor_reduce(
                        out=BMINS[:, b0:b0 + KB], in_=v4,
                        axis=mybir.AxisListType.X, op=mybir.AluOpType.min,
                    )

            nc.sync.dma_start(out=res[:, 0:n_b], in_=BMINS[:, :])
            nc.sync.dma_start(out=res[:, n_b:n_b + 2 * n_a], in_=AMINS[:, :])

    import json as _json

    pruned = _prune_redundant_waits(_json.loads(nc.to_json_bytes()))
    blob = _json.dumps(pruned).encode()
    nc.to_json_bytes = lambda: blob  # instance override read by bass2jax
    return nc


def _host_prep(recon_points: np.ndarray, gt_points: np.ndarray):
    """Build the per-core [128, ABCOLS] bf16 operand."""
    in_maps = []
    for core in range(N_CORES):
        b, h = divmod(core, 2)
        ii = _II[h * HALF:(h + 1) * HALF]
        jj = _JJ[h * HALF:(h + 1) * HALF]
        rec = recon_points[b].astype(np.float32)          # [64, 3]
        start, end = rec[ii], rec[jj]                     # [1008, 3]

        A = np.zeros((5, ROWS), dtype=np.float32)
        for fi, f in enumerate(FRACS):
            k = (start * np.float32(f) + end * np.float32(1.0 - f)).astype(np.float32)
            cols = slice(fi * HALF, (fi + 1) * HALF)
            A[0:3, cols] = k.T
            A[3, cols] = (k.astype(np.float64) ** 2).sum(1).astype(np.float32)
            A[4, cols] = 1.0
        ep = slice(NINT, NINT + NEND)
        re = rec[32 * h:32 * h + 32]
        A[0:3, ep] = re.T
        A[3, ep] = (re.astype(np.float64) ** 2).sum(1).astype(np.float32)
        A[4, ep] = 1.0

        g = gt_points[b].astype(np.float32)               # [2048, 3]
        Bm = np.empty((5, M), dtype=np.float32)
        Bm[0:3] = np.float32(-2.0) * g.T
        Bm[3] = 1.0
        Bm[4] = (g.astype(np.float64) ** 2).sum(1).astype(np.float32)

        Ah, Al, Ar = _split3_bf16(A)
        Bh, Bl, Br = _split3_bf16(Bm)
        # Product groups, largest magnitude first: hh | hl lh | hr rh ll
        A_ext = np.concatenate([Ah, Ah, Al, Ah, Ar, Al], axis=0)  # [30, ROWS]
        B_ext = np.concatenate([Bh, Bl, Bh, Br, Bh, Bl], axis=0)  # [30, M]

        import ml_dtypes
        bf16 = ml_dtypes.bfloat16
        ab = np.zeros((128, ABCOLS), dtype=bf16)
        for q in range(4):
            ab[32 * q:32 * q + 30, 0:M] = B_ext
        # Pair tiles: even t on partition bands 0 and 1, odd t on 2 and 3,
        # duplicated at both offsets its two halves use.
        for t in range(NTILES):
            colb = M + (t // 2) * 128
            base = 64 * (t % 2)
            blk = A_ext[:, 128 * t:128 * (t + 1)]
            ab[base:base + 30, colb:colb + 128] = blk
            ab[base + 32:base + 62, colb:colb + 128] = blk
        in_maps.append({"ab": np.ascontiguousarray(ab)})
    return in_maps


def _host_assemble(results) -> np.ndarray:
    n_a = len(A_SET)
    n_b = NTILES - n_a
    out = np.empty((B, P), dtype=np.float32)
    E_all = np.empty((B, NPTS), dtype=np.float32)
    s3 = {}
    for core in range(N_CORES):
        b, h = divmod(core, 2)
        res = np.asarray(results[core]["res"], dtype=np.float32)  # [128, 30]
        # res columns: [0:n_b] = B-tile mins (b-ordinal), [n_b:] = A-tile
        # half-mins (two per tile).
        tmins = np.empty((128, NTILES), dtype=np.float32)
        a_idx = b_idx = 0
        for t in range(NTILES):
            if t in A_SET:
                tmins[:, t] = np.minimum(res[:, n_b + 2 * a_idx],
                                         res[:, n_b + 2 * a_idx + 1])
                a_idx += 1
            else:
                tmins[:, t] = res[:, b_idx]
                b_idx += 1
        mins = tmins.T.reshape(-1)                # row r = 128*t + p
        s3[(b, h)] = mins[0:NINT].reshape(NF, HALF).sum(axis=0)
        E_all[b, 32 * h:32 * h + 32] = mins[NINT:NINT + NEND]
    for b in range(B):
        E = E_all[b]
        for h in range(2):
            sl = slice(h * HALF, (h + 1) * HALF)
            out[b, sl] = (s3[(b, h)] + E[_II[sl]] + E[_JJ[sl]]) * np.float32(0.2)
    return out


_NC_CACHE = None


def _get_nc() -> bass.Bass:
    global _NC_CACHE
    if _NC_CACHE is None:
        _NC_CACHE = _build_nc()
    return _NC_CACHE


def run(recon_points: np.ndarray, gt_points: np.ndarray, **spmd_kwargs):
    """Run on 8 NeuronCores; returns (output [4, 2016], BassKernelResults)."""
    nc = _get_nc()
    in_maps = _host_prep(recon_points, gt_points)
    r = run_bass_kernel_spmd(nc, in_maps, list(range(N_CORES)), **spmd_kwargs)
    return _host_assemble(r.results), r


def kernel(recon_points: np.ndarray, gt_points: np.ndarray) -> np.ndarray:
    recon_points = np.asarray(recon_points, dtype=np.float32)
    gt_points = np.asarray(gt_points, dtype=np.float32)
    out, _ = run(recon_points, gt_points)
    return out


# revision 41
# speedup vs baseline: 1.0433x; 1.0433x over previous
"""Trainium2 Bass kernel for nn_ComputeEdgeLoss.

Computes, for each batch b and lower-triangular pair (i, j) of the 64
recon keypoints, the mean over 5 interpolated segment points of the min
squared distance to the 2048 gt points of that batch.

Strategy (v2.1)
---------------
Sharding: 8 cores = 4 batches x 2 pair-halves (1008 pairs each); the 64
endpoint queries of a batch are split 32/32 between its two cores, so
each core owns exactly 3056 query rows -> 24 row-tiles of 128 (the
baseline's 25).  gt replicated per batch (sharding_hint).

Math: for a query point k and gt point g,
    ||k - g||^2 = a . b,  a = [kx, ky, kz, ||k||^2, 1],
                          b = [-2gx, -2gy, -2gz, 1, ||g||^2]
so one PE matmul produces a [128 x 512] block of squared distances in
PSUM.  Precision: fp32 inputs are split into three bf16 terms
x ~= h + l + r and the six product groups >= 2^-24 (hh, hl, lh, hr, rh,
ll) become K=30 bf16 contraction rows (padded to 32).  Measured on the
real data this lands at ~3e-4 max rel err vs the 2e-2 gate.

PE array packing: K=32 <= 32 enables 4x row tiling -- the 128x128 PE
splits into four 32x128 bands (tile_position=(32q, 0)), each holding a
different stationary pair-block and streaming its own gt chunk, so four
matmuls run concurrently (~3x measured PE throughput).  Operands for
band q live on SBUF partitions [32q, 32q+32): gt is replicated at all
four offsets, pair tiles at the two offsets their halves use.

Reduction (the bottleneck): this walrus rejects every raw-ISA DVE
instruction ("ISA wrong length" in visitInstISA) -- no custom DVE ops,
no InstTensorTensorReduce -- so the drain uses only classic BIR ops:
 - A-tiles: DVE fp32 tensor_reduce(min) straight from PSUM (1 el/cyc);
 - B-tiles: ScalarE casts the [128, 2048] tile to fp16 in SBUF
   (1 el/cyc), DVE folds KB=3 staged tiles with 2x-rate fp16
   tensor_tensor(min) chains + one batched tensor_reduce.
6 A + 18 B balances DVE (~34us) against ScalarE (~33us); PE ~7-14us
is fully hidden behind the drain.

v2.1 deltas (structure-preserving): the A-tile of each group of 4 comes
FIRST (A_SET at 0,4,...) so the DVE has PSUM work as soon as tile 0
lands instead of idling ~16us behind three ScalarE copies; and a 1-col
warmup copy pulls the one-time ~1.3us ACT table load off the ScalarE
critical path.
"""

import numpy as np

import concourse.bass as bass
import concourse.mybir as mybir
import concourse.tile as tile
from concourse.bass_utils import run_bass_kernel_spmd

# Problem shape (hardcoded per contest rules).
B = 4          # batches
NPTS = 64      # recon points per batch
M = 2048       # gt points per batch
P = NPTS * (NPTS - 1) // 2   # 2016 pairs
HALF = P // 2                # 1008 pairs per core
N_CORES = 8
FRACS = (0.25, 0.5, 0.75)    # interior interpolation fractions
NF = len(FRACS)
NINT = NF * HALF             # 3024 interior rows per core
NEND = 32                    # endpoint rows per core (64 split 32/32)
NTILES = 24                  # row-tiles of 128 (3072 rows, 3056 used)
ROWS = NTILES * 128
KEXT = 32                    # padded contraction depth (30 real rows)
PFCOLS = (NTILES // 2) * 128  # 1536: even tiles on partitions 0-63, odd on 64-127
ABCOLS = M + PFCOLS

_II, _JJ = np.tril_indices(NPTS, -1)   # pair order matches reference

# Tiles drained by DVE directly (A); the rest are ScalarE-staged (B).
# Pattern B,A,B,B per group of 4: tile 0 is a B so ScalarE's first copy
# starts as soon as the FIRST tile lands (the end time is pinned by
# ScalarE's finish + the last fold chain), while tile 1 is an A so the
# DVE also has PSUM work almost immediately; the staged copies then
# chain back-to-back on ScalarE and the DVE folds run one group behind.
A_SET = (1, 5, 9, 13, 17, 21)
KB = 3                       # B-tiles per batched DVE fold chain


def _split3_bf16(x: np.ndarray):
    """Split fp32 x into three bf16 terms with x ~= h + l + r (27-bit
    significand fidelity; differences are Sterbenz-exact in fp32)."""
    import ml_dtypes

    bf16 = ml_dtypes.bfloat16
    x = np.ascontiguousarray(x, dtype=np.float32)
    h = x.astype(bf16)
    l32 = (x - h.astype(np.float32)).astype(np.float32)
    l = l32.astype(bf16)
    r = (l32 - l.astype(np.float32)).astype(np.float32).astype(bf16)
    return h, l, r


_COMPUTE_ENGINES = {"PE", "DVE", "Activation", "Pool"}


def _prune_redundant_waits(bir: dict) -> dict:
    """Reduce every instruction to at most ONE sync-wait.

    This walrus build accepts only one sync-wait per instruction, but
    Tile's semaphore pass is not transitively minimal.  We reconstruct
    per-instruction guaranteed semaphore lower bounds (vector clocks
    over the scheduled program order) and delete implied waits; any
    residual multi-wait instruction is split into single-wait Drain
    carriers on the same engine.

    Soundness model: per-engine in-order dispatch; in-order completion
    for compute engines; per-semaphore in-order completion for DMA-queue
    sems (each DMAHW sem belongs to one queue).  Only monotone
    (inc-only) semaphores with sem-ge-imm waits participate.
    """
    fn = bir["functions"][0]

    # Semaphore properties across the whole program.
    contrib_engines: dict[int, set] = {}
    monotone: dict[int, bool] = {}
    for b in fn["blocks"]:
        for ins in b["instructions"]:
            sy = ins.get("sync_info") or {}
            for u in sy.get("on_update") or []:
                if u.get("sync_type") != "semaphore":
                    continue
                s = u["id"]
                contrib_engines.setdefault(s, set()).add(ins.get("engine"))
                ok = u.get("update_mode") == "sem-inc"
                monotone[s] = monotone.get(s, True) and ok

    def usable(s):
        return monotone.get(s, False)

    def mergemax(dst, src):
        for k, v in src.items():
            if dst.get(k, -1) < v:
                dst[k] = v

    prev_start_know: dict[str, dict] = {}
    cum: dict[int, int] = {}            # sem -> cumulative inc in walk order
    comp_know: list[dict] = []          # per walk index
    sem_reach: dict[int, list] = {}     # sem -> [(value_after, walk_idx)]
    dropped = 0
    walk_idx = 0

    for b in fn["blocks"]:
        new_insts = []
        for ins in b["instructions"]:
            eng = ins.get("engine")
            sy = ins.get("sync_info") or {}
            waits = list(sy.get("on_wait") or [])

            def know_from(wlist):
                know = dict(prev_start_know.get(eng, {}))
                for w in wlist:
                    if (w.get("sync_type") != "semaphore"
                            or w.get("wait_mode") != "sem-ge-imm"):
                        continue
                    s, v = w["id"], w["wait_value"]
                    if not usable(s):
                        continue
                    if know.get(s, -1) < v:
                        know[s] = v
                    if len(contrib_engines.get(s, ())) == 1:
                        for after, pidx in sem_reach.get(s, ()):
                            if after >= v:
                                mergemax(know, comp_know[pidx])
                                break
                return know

            if len(waits) > 1:
                kept = list(waits)
                changed = True
                while changed and len(kept) > 1:
                    changed = False
                    for w in list(kept):
                        others = [x for x in kept if x is not w]
                        if (w.get("sync_type") == "semaphore"
                                and w.get("wait_mode") == "sem-ge-imm"
                                and usable(w["id"])
                                and know_from(others).get(w["id"], -1)
                                >= w["wait_value"]):
                            kept.remove(w)
                            dropped += 1
                            changed = True
                            break
                if len(kept) > 1:
                    # Split: carrier Drains each take one wait.
                    for k, w in enumerate(kept[:-1]):
                        new_insts.append({
                            "name": f"{ins['name']}-w{k}",
                            "engine": eng, "ins": [], "outs": [],
                            "opcode": "Drain",
                            "sync_info": {"on_wait": [w], "on_update": []},
                        })
                        walk_idx += 1
                        comp_know.append(dict(prev_start_know.get(eng, {})))
                    kept = kept[-1:]
                if len(kept) != len(waits):
                    if not sy:
                        ins["sync_info"] = sy = {"on_update": []}
                    sy["on_wait"] = kept
                    waits = kept

            start_know = know_from(waits)
            prev_start_know[eng] = start_know

            own = set()
            for u in sy.get("on_update") or []:
                if (u.get("sync_type") == "semaphore"
                        and u.get("update_mode") == "sem-inc"):
                    s = u["id"]
                    cum[s] = cum.get(s, 0) + u.get("update_value", 1)
                    sem_reach.setdefault(s, []).append((cum[s], walk_idx))
                    own.add(s)
            ck = dict(start_know)
            for s in own:
                if usable(s) and len(contrib_engines.get(s, ())) == 1:
                    if ck.get(s, -1) < cum[s]:
                        ck[s] = cum[s]
            if eng in _COMPUTE_ENGINES:
                for s, c in cum.items():
                    if (usable(s) and contrib_engines.get(s) == {eng}
                            and ck.get(s, -1) < c):
                        ck[s] = c
            comp_know.append(ck)
            new_insts.append(ins)
            walk_idx += 1
        b["instructions"] = new_insts
    return bir


def _build_nc() -> bass.Bass:
    nc = bass.Bass()
    # Single fused input tensor: gt (replicated per 32-partition band)
    # in cols [0, M), pair tiles in cols [M, ABCOLS).
    ab = nc.declare_dram_parameter("ab", [128, ABCOLS], mybir.dt.bfloat16,
                                   isOutput=False)
    n_a = len(A_SET)
    n_b = NTILES - n_a
    assert n_b % KB == 0
    res = nc.declare_dram_parameter("res", [128, NTILES + len(A_SET)],
                                    mybir.dt.float32, isOutput=True)

    f32 = mybir.dt.float32
    bf16 = mybir.dt.bfloat16
    f16 = mybir.dt.float16

    with tile.TileContext(nc) as tc:
        with (
            tc.tile_pool(name="const", bufs=1) as const_pool,
            tc.tile_pool(name="psum", bufs=4, space="PSUM") as psum_pool,
            tc.tile_pool(name="cp", bufs=2) as cp_pool,
            tc.tile_pool(name="fold", bufs=2) as fold_pool,
        ):
            AB = const_pool.tile([128, ABCOLS], bf16, name="AB")
            AMINS = const_pool.tile([128, 2 * n_a], f32, name="AMINS")
            BMINS = const_pool.tile([128, n_b], f32, name="BMINS")
            import os as _os
            _ww = 1 + int(_os.environ.get("KERNEL_SALT", "0"))
            WARM = const_pool.tile([128, _ww], f32, name="WARM")

            # Pull the one-time ~1.3us ACT table load off the ScalarE
            # critical path: a 1-col warmup copy right after the engine
            # preamble, overlapping the input DMA.
            nc.gpsimd.memset(WARM[:, :], 0.0)
            nc.scalar.copy(WARM[:, :], WARM[:, :])

            # Full-partition-width loads (partition-sliced DMAs are slow),
            # ordered so tile 0's operands (gt lo-half + first pf block)
            # land first and its matmuls start as early as possible.
            PF0 = M + 4 * 128     # pf tiles 0..7 boundary
            nc.sync.dma_start(out=AB[:, 0:M // 2], in_=ab[:, 0:M // 2])
            nc.sync.dma_start(out=AB[:, M:PF0], in_=ab[:, M:PF0])
            nc.sync.dma_start(out=AB[:, M // 2:M], in_=ab[:, M // 2:M])
            nc.sync.dma_start(out=AB[:, PF0:ABCOLS], in_=ab[:, PF0:ABCOLS])

            a_idx = 0
            b_idx = 0
            cp_cur = None
            HM = M // 2
            for t in range(NTILES):
                col = M + (t // 2) * 128
                halves = []
                for hh in range(2):
                    q = (2 * t + hh) % 4
                    stat = AB[32 * q:32 * q + 32, col:col + 128]
                    ptile = psum_pool.tile([128, HM], f32, tag="ptile")
                    halves.append(ptile)
                    for c in range(2):
                        sl_g = slice(HM * hh + 512 * c, HM * hh + 512 * (c + 1))
                        nc.tensor.matmul(
                            out=ptile[:, 512 * c:512 * (c + 1)],
                            lhsT=stat, rhs=AB[32 * q:32 * q + 32, sl_g],
                            start=True, stop=True,
                            tile_position=(32 * q, 0),
                        )
                if t in A_SET:
                    for hh in range(2):
                        nc.vector.tensor_reduce(
                            out=AMINS[:, 2 * a_idx + hh:2 * a_idx + hh + 1],
                            in_=halves[hh][:, :],
                            axis=mybir.AxisListType.X, op=mybir.AluOpType.min,
                        )
                    a_idx += 1
                    continue

                j = b_idx % KB
                if j == 0:
                    cp_cur = cp_pool.tile([128, KB * M], f16, tag="cp")
                for hh in range(2):
                    nc.scalar.copy(
                        cp_cur[:, j * M + hh * HM:j * M + (hh + 1) * HM],
                        halves[hh][:, :])
                b_idx += 1
                if j == KB - 1:
                    b0 = b_idx - KB
                    c3 = cp_cur[:, :].rearrange("p (k n) -> p k n", n=M)
                    j1 = fold_pool.tile([128, KB * (M // 2)], f16, tag="j1")
                    v1 = j1[:, :].rearrange("p (k n) -> p k n", n=M // 2)
                    nc.vector.tensor_tensor(
                        out=v1, in0=c3[:, :, 0:M // 2], in1=c3[:, :, M // 2:M],
                        op=mybir.AluOpType.min)
                    j2 = fold_pool.tile([128, KB * (M // 4)], f16, tag="j2")
                    v2 = j2[:, :].rearrange("p (k n) -> p k n", n=M // 4)
                    nc.vector.tensor_tensor(
                        out=v2, in0=v1[:, :, 0:M // 4], in1=v1[:, :, M // 4:M // 2],
                        op=mybir.AluOpType.min)
                    j3 = fold_pool.tile([128, KB * (M // 8)], f16, tag="j3")
                    v3 = j3[:, :].rearrange("p (k n) -> p k n", n=M // 8)
                    nc.vector.tensor_tensor(
                        out=v3, in0=v2[:, :, 0:M // 8], in1=v2[:, :, M // 8:M // 4],
                        op=mybir.AluOpType.min)
                    j4 = fold_pool.tile([128, KB * (M // 16)], f16, tag="j4")
                    v4 = j4[:, :].rearrange("p (k n) -> p k n", n=M // 16)
                    nc.vector.tensor_tensor(
                        out=v4, in0=v3[:, :, 0:M // 16], in1=v3[:, :, M // 16:M // 8],
                        op=mybir.AluOpType.min)
                    nc.vector.tensor_reduce(
                        out=BMINS[:, b0:b0 + KB], in_=v4,
                        axis=mybir.AxisListType.X, op=mybir.AluOpType.min,
                    )

            nc.sync.dma_start(out=res[:, 0:n_b], in_=BMINS[:, :])
            nc.sync.dma_start(out=res[:, n_b:n_b + 2 * n_a], in_=AMINS[:, :])

    import json as _json

    pruned = _prune_redundant_waits(_json.loads(nc.to_json_bytes()))
    blob = _json.dumps(pruned).encode()
    nc.to_json_bytes = lambda: blob  # instance override read by bass2jax
    return nc


def _host_prep(recon_points: np.ndarray, gt_points: np.ndarray):
    """Build the per-core [128, ABCOLS] bf16 operand."""
    in_maps = []
    for core in range(N_CORES):
        b, h = divmod(core, 2)
        ii = _II[h * HALF:(h + 1) * HALF]
        jj = _JJ[h * HALF:(h + 1) * HALF]
        rec = recon_points[b].astype(np.float32)          # [64, 3]
        start, end = rec[ii], rec[jj]                     # [1008, 3]

        A = np.zeros((5, ROWS), dtype=np.float32)
        for fi, f in enumerate(FRACS):
            k = (start * np.float32(f) + end * np.float32(1.0 - f)).astype(np.float32)
            cols = slice(fi * HALF, (fi + 1) * HALF)
            A[0:3, cols] = k.T
            A[3, cols] = (k.astype(np.float64) ** 2).sum(1).astype(np.float32)
            A[4, cols] = 1.0
        ep = slice(NINT, NINT + NEND)
        re = rec[32 * h:32 * h + 32]
        A[0:3, ep] = re.T
        A[3, ep] = (re.astype(np.float64) ** 2).sum(1).astype(np.float32)
        A[4, ep] = 1.0

        g = gt_points[b].astype(np.float32)               # [2048, 3]
        Bm = np.empty((5, M), dtype=np.float32)
        Bm[0:3] = np.float32(-2.0) * g.T
        Bm[3] = 1.0
        Bm[4] = (g.astype(np.float64) ** 2).sum(1).astype(np.float32)

        Ah, Al, Ar = _split3_bf16(A)
        Bh, Bl, Br = _split3_bf16(Bm)
        # Product groups, largest magnitude first: hh | hl lh | hr rh ll
        A_ext = np.concatenate([Ah, Ah, Al, Ah, Ar, Al], axis=0)  # [30, ROWS]
        B_ext = np.concatenate([Bh, Bl, Bh, Br, Bh, Bl], axis=0)  # [30, M]

        import ml_dtypes
        bf16 = ml_dtypes.bfloat16
        ab = np.zeros((128, ABCOLS), dtype=bf16)
        for q in range(4):
            ab[32 * q:32 * q + 30, 0:M] = B_ext
        # Pair tiles: even t on partition bands 0 and 1, odd t on 2 and 3,
        # duplicated at both offsets its two halves use.
        for t in range(NTILES):
            colb = M + (t // 2) * 128
            base = 64 * (t % 2)
            blk = A_ext[:, 128 * t:128 * (t + 1)]
            ab[base:base + 30, colb:colb + 128] = blk
            ab[base + 32:base + 62, colb:colb + 128] = blk
        in_maps.append({"ab": np.ascontiguousarray(ab)})
    return in_maps


def _host_assemble(results) -> np.ndarray:
    n_a = len(A_SET)
    n_b = NTILES - n_a
    out = np.empty((B, P), dtype=np.float32)
    E_all = np.empty((B, NPTS), dtype=np.float32)
    s3 = {}
    for core in range(N_CORES):
        b, h = divmod(core, 2)
        res = np.asarray(results[core]["res"], dtype=np.float32)  # [128, 30]
        # res columns: [0:n_b] = B-tile mins (b-ordinal), [n_b:] = A-tile
        # half-mins (two per tile).
        tmins = np.empty((128, NTILES), dtype=np.float32)
        a_idx = b_idx = 0
        for t in range(NTILES):
            if t in A_SET:
                tmins[:, t] = np.minimum(res[:, n_b + 2 * a_idx],
                                         res[:, n_b + 2 * a_idx + 1])
                a_idx += 1
            else:
                tmins[:, t] = res[:, b_idx]
                b_idx += 1
        mins = tmins.T.reshape(-1)                # row r = 128*t + p
        s3[(b, h)] = mins[0:NINT].reshape(NF, HALF).sum(axis=0)
        E_all[b, 32 * h:32 * h + 32] = mins[NINT:NINT + NEND]
    for b in range(B):
        E = E_all[b]
        for h in range(2):
            sl = slice(h * HALF, (h + 1) * HALF)
            out[b, sl] = (s3[(b, h)] + E[_II[sl]] + E[_JJ[sl]]) * np.float32(0.2)
    return out


_NC_CACHE = None


def _get_nc() -> bass.Bass:
    global _NC_CACHE
    if _NC_CACHE is None:
        _NC_CACHE = _build_nc()
    return _NC_CACHE


def run(recon_points: np.ndarray, gt_points: np.ndarray, **spmd_kwargs):
    """Run on 8 NeuronCores; returns (output [4, 2016], BassKernelResults)."""
    nc = _get_nc()
    in_maps = _host_prep(recon_points, gt_points)
    r = run_bass_kernel_spmd(nc, in_maps, list(range(N_CORES)), **spmd_kwargs)
    return _host_assemble(r.results), r


def kernel(recon_points: np.ndarray, gt_points: np.ndarray) -> np.ndarray:
    recon_points = np.asarray(recon_points, dtype=np.float32)
    gt_points = np.asarray(gt_points, dtype=np.float32)
    out, _ = run(recon_points, gt_points)
    return out
